# revision 33
# baseline (speedup 1.0000x reference)
"""MultiHeadAttention kernel for Trainium2, 8-core hybrid batch x head sharding.

Problem: S=2048, B=2, D=1024, 16 heads of d=64 (batch_first=False).
Sharding: core c handles batch b=c//4 and head group hg=c%4 (4 heads =
256 output dims), processed as 2 "pairs" of 2 heads (a pair = 128
partitions = 2x64 head dims).

v2 schedule (vs v1): the exp stream is the critical engine (~147us of
ScalarE work); the v1 loss was ~46us of ACT gaps during the input DMA
window. Changes:
  - global exp order interleaves pairs per i-chunk:
    (p0,ic0),(p1,ic0),(p0,ic1),... so the first 32 exps need only
    q block 0; the DMA stream is ordered [wq,xq0,wk,xk0,k1,k2,k3,
    wv,v0..v3,q1,q2,q3] to match first-need times (k blocks gate the
    exp stream hardest, q blocks 1-3 are needed latest).
  - at_t ring enlarged to 20 slots so pv can trail the exps by up to
    18 j-tiles early on (v blocks arrive late in the DMA stream);
    the lag tapers back to 2 once all v data is resident.
  - PSUM: scores double-buffered 2x[128,1024] (4 banks), pv 2 banks,
    projection scratch 2 banks (warm chain shares its tag).
  - a PE warm chain bridges the initial DMA wait so projections run
    at 2.4GHz (HAM warm) from the start.
Everything else (ones-column softmax denominator inside pv, DVE-only
normalization, DMA-xbar v transpose) is unchanged from v1.
"""

import sys

if "/opt/trn_rl_repo" not in sys.path:
    sys.path.insert(0, "/opt/trn_rl_repo")

import numpy as np
import ml_dtypes

import concourse.bass as bass
import concourse.mybir as mybir
import concourse.tile as tile
from concourse import bacc

BF16 = mybir.dt.bfloat16
FP32 = mybir.dt.float32
FP8 = mybir.dt.float8e4
NP_BF16 = ml_dtypes.bfloat16

D = 1024
NHEAD = 16
DH = 64
NCORES = 8
S = 2048
B = 2
HPC = 4                      # heads per core
DC = HPC * DH                # per-core output dims = 256
NPAIR = 2                    # head pairs per core (128 dims each)
KT = D // 128                # contraction tiles = 8
TB = 512                     # token block for projections
NTB = S // TB                # 4
IC = 512                     # i-chunk width
NIC = S // IC                # 4
JT = S // 128                # j-tiles = 16
RING = 22                    # at_t ring slots
NEXP = NPAIR * NIC * JT      # 128 global exp units
SCALE = 1.0 / float(np.sqrt(DH))
# constant subtracted inside the exp so attention weights fit fp8e4m3
# (numerator and denominator of the softmax scale by the same e^-C, so
# the output is unchanged; scores*SCALE is ~N(8, 1.7) on this data)
EXP_SHIFT = -12.0


def build_program():
    nc = bacc.Bacc(
        "TRN2", target_bir_lowering=False, debug=False, num_devices=NCORES
    )
    xq = nc.dram_tensor("xq", [NTB, 128, KT, TB], BF16, kind="ExternalInput")
    xk = nc.dram_tensor("xk", [NTB, 128, KT, TB], BF16, kind="ExternalInput")
    xv = nc.dram_tensor("xv", [NTB, 128, KT, TB], BF16, kind="ExternalInput")
    wq = nc.dram_tensor("wq", [128, KT, DC], BF16, kind="ExternalInput")
    wk = nc.dram_tensor("wk", [128, KT, DC], BF16, kind="ExternalInput")
    wv = nc.dram_tensor("wv", [128, KT, DC], BF16, kind="ExternalInput")
    bqkv = nc.dram_tensor("bqkv", [128, NPAIR, 3], FP32, kind="ExternalInput")
    out = nc.dram_tensor("out", [DC, S], FP32, kind="ExternalOutput")

    with tile.TileContext(nc) as tc:
        with (
            tc.tile_pool(name="const", bufs=1) as constp,
            tc.tile_pool(name="xin", bufs=1) as xp,
            tc.tile_pool(name="qkv", bufs=1) as qkvp,
            tc.tile_pool(name="vstg", bufs=2) as vstgp,
            tc.tile_pool(name="attn", bufs=1) as atp,
            tc.tile_pool(name="outp", bufs=2) as outp,
            tc.tile_pool(name="drain", bufs=2) as drainp,
            tc.tile_pool(name="sc", bufs=2, space="PSUM") as scp,
            tc.tile_pool(name="prj", bufs=2, space="PSUM") as prjp,
            tc.tile_pool(name="pv", bufs=2, space="PSUM") as pvp,
        ):
            # ---- input loads, all on the SWDGE (gpsimd) queue, in
            # first-need order: prologue (wq,xq0,wk,xk0), then all k
            # blocks, then v, then the remaining q blocks.
            xq_t = xp.tile([128, NTB, KT, TB], BF16, tag="xq")
            xk_t = xp.tile([128, NTB, KT, TB], BF16, tag="xk")
            xv_t = xp.tile([128, NTB, KT, TB], BF16, tag="xv")
            xts = {0: xq_t, 1: xk_t, 2: xv_t}
            xsrc = {0: xq, 1: xk, 2: xv}
            wq_t = constp.tile([128, KT, DC], BF16, tag="wq")
            wk_t = constp.tile([128, KT, DC], BF16, tag="wk")
            wv_t = constp.tile([128, KT, DC], BF16, tag="wv")

            # Input stream: 12 transfers on the gpsimd SWDGE queue (its
            # descriptor ring is ~12 deep), in first-need order, with the
            # critical prefix minimized: only the pair-0 weight columns and
            # the first 128 k tokens come before the first exp can fire
            # (the whole-chip prefix traffic is what gates the start - all
            # 8 cores pull simultaneously). Pair-1 weights and q blocks 1-3
            # ride the scalar HWDGE queue, pushed between early exps; wv
            # rides sync.
            def dma_x(q, kind, tb):
                q.dma_start(out=xts[kind][:, tb], in_=xsrc[kind][tb])

            nc.gpsimd.dma_start(out=wq_t[:, :, 0:128], in_=wq[:, :, 0:128])
            dma_x(nc.gpsimd, 0, 0)
            nc.gpsimd.dma_start(out=wk_t[:, :, 0:128], in_=wk[:, :, 0:128])
            # xk block 0 split: tokens 0-127 unlock the first scores tile
            nc.gpsimd.dma_start(out=xk_t[:, 0, :, 0:128], in_=xk[0][:, :, 0:128])
            nc.gpsimd.dma_start(out=xk_t[:, 0, :, 128:TB], in_=xk[0][:, :, 128:TB])
            for tb in range(1, NTB):
                dma_x(nc.gpsimd, 1, tb)
            for tb in range(NTB):
                dma_x(nc.gpsimd, 2, tb)
            nc.sync.dma_start(out=wv_t[:], in_=wv[:, :, :])

            # ---- exp table preload: fire ACT_TABLE_LOAD during the DMA phase
            pre_in = constp.tile([128, 16], FP32, tag="prei")
            nc.vector.memset(pre_in[:], 0.0)
            pre_out = constp.tile([128, 16], BF16, tag="preo")
            nc.scalar.activation(
                out=pre_out[:], in_=pre_in[:],
                func=mybir.ActivationFunctionType.Exp, scale=1.0,
            )

            # ---- constants
            bqkv_t = constp.tile([128, NPAIR, 3], FP32, tag="bqkv")
            nc.sync.dma_start(out=bqkv_t[:], in_=bqkv[:, :, :])
            shift_t = constp.tile([128, 1], FP32, tag="shift")
            nc.vector.memset(shift_t[:], EXP_SHIFT)

            # ---- persistent activations
            q_t = qkvp.tile([128, NPAIR, S], BF16, tag="q")
            k_t = qkvp.tile([128, NPAIR, S], BF16, tag="k")
            vx_t = qkvp.tile([128, NPAIR, JT, 2 * DH], BF16, tag="vx")
            # per head: cols 0:64 = projected v, cols 64:128 = ones, so the
            # pv matmul replicates the softmax denominator across output
            # partitions 64:128. fp8 so pv can run in DoubleRow perf mode
            # (2 j-tiles per matmul - halves the pv matmul count).
            v_t = qkvp.tile([128, NPAIR, JT, 2, 2 * DH], BF16, tag="v")
            nc.vector.memset(v_t[:, :, :, :, DH : 2 * DH], 1.0)
            # attention-weights ring indexed by global exp unit % RING
            at_t = atp.tile([128, RING, 2 * IC], BF16, tag="at", bufs=1)

            wts = {0: wq_t, 1: wk_t, 2: wv_t}

            # ---- HAM warm chain: identical-weights matmuls chained on wq
            # keep the PE busy from wq arrival through the xk0 DMA wait so
            # the first projections run at 2.4GHz. Shares the proj scratch
            # tag (it is write-only; the first proj unit simply WARs on it).
            warm = prjp.tile([128, TB], FP32, tag="prj", name="warm")
            for _ in range(40):
                nc.tensor.matmul(
                    warm[:, 0:128], wq_t[:, 0, 0:128], wq_t[:, 0, 0:128],
                    start=True, stop=True,
                )

            proj_ps = {}
            emitted_units = set()   # (kind, tb, p) whose drain has been emitted

            def emit_proj_mm(kind, tb, p, half):
                if half == 1:
                    emitted_units.add((kind, tb, p))
                # one half of a projection's K-accumulation: 4 matmuls
                x_t = xts[kind]
                w_t = wts[kind]
                if half == 0:
                    proj_ps[(kind, tb, p)] = prjp.tile(
                        [128, TB], FP32, tag="prj", name=f"ps{kind}{tb}{p}"
                    )
                ps = proj_ps[(kind, tb, p)]
                for kt in range(half * 4, half * 4 + 4):
                    nc.tensor.matmul(
                        ps[:, :], w_t[:, kt, p * 128 : (p + 1) * 128],
                        x_t[:, tb, kt, :],
                        start=(kt == 0), stop=(kt == KT - 1),
                    )
                if half == 1:
                    bias = bqkv_t[:, p, kind : kind + 1].to_broadcast((128, TB))
                    if kind < 2:
                        dst = q_t if kind == 0 else k_t
                        nc.vector.tensor_add(
                            dst[:, p, tb * TB : (tb + 1) * TB], ps[:, :], bias
                        )
                    else:
                        vTt = vstgp.tile([128, TB], BF16, tag="vT", name="vT")
                        nc.vector.tensor_add(vTt[:, :], ps[:, :], bias)
                        j0 = tb * (TB // 128)
                        j1 = (tb + 1) * (TB // 128)
                        nc.sync.dma_start_transpose(vx_t[:, p, j0:j1, :], vTt[:, :])
                        nc.vector.tensor_copy(
                            v_t[:, p, j0:j1, :, 0:DH],
                            vx_t[:, p, j0:j1, :].rearrange("t j (h d) -> t j h d", h=2),
                        )
                    del proj_ps[(kind, tb, p)]

            def emit_proj(kind, tb, p):
                emit_proj_mm(kind, tb, p, 0)
                emit_proj_mm(kind, tb, p, 1)

            # prologue: q block 0 pair 0 (full), then k block 0 pair 0 in
            # two pieces - tokens 0:128 first (they unlock the first scores
            # tile as soon as the quarter-block xk0a DMA lands), then the
            # rest
            emit_proj(0, 0, 0)
            k00ps = prjp.tile([128, TB], FP32, tag="prj", name="k00ps")
            kbias = bqkv_t[:, 0, 1:2]
            for lo, hi in ((0, 128), (128, TB)):
                for kt in range(KT):
                    nc.tensor.matmul(
                        k00ps[:, lo:hi], wk_t[:, kt, 0:128],
                        xk_t[:, 0, kt, lo:hi],
                        start=(kt == 0), stop=(kt == KT - 1),
                    )
                nc.vector.tensor_add(
                    k_t[:, 0, lo:hi], k00ps[:, lo:hi],
                    kbias.to_broadcast((128, hi - lo)),
                )
            emitted_units.add((1, 0, 0))

            # ---- deadline-scheduled projection units, half-projection
            # granularity. point = global exp index (see SWEEPS below).
            def _scoped(name, fn):
                def g():
                    with nc.named_scope(name):
                        fn()
                return g

            def proj_halves(pt0, pt1, kind, tb, p):
                n = f"U{'qkv'[kind]}{tb}p{p}"
                return [
                    (pt0, _scoped(n + "a", lambda: emit_proj_mm(kind, tb, p, 0))),
                    (pt1, _scoped(n + "b", lambda: emit_proj_mm(kind, tb, p, 1))),
                ]

            # v_h1_pt[p][tb] = injection point of the v(tb,p) unit's second
            # half (whose drain writes v_t); pv units must be emitted AFTER
            # that point or they read uninitialized v_t (build asserts below)
            v_h1_pt = {0: [20, 24, 29, 33], 1: [27, 36, 41, 45]}
            units = (
                # k blocks 1-3 pair 0 (needed at exp #4tb) and all pair-1
                # k/q-block-0 projections go in the early, DMA-stalled
                # window where the PE has slack
                proj_halves(1, 2, 1, 1, 0)
                + proj_halves(4, 5, 1, 2, 0)
                + proj_halves(7, 8, 1, 3, 0)
                + proj_halves(9, 10, 0, 0, 1)
                + proj_halves(11, 12, 1, 0, 1)
                + proj_halves(13, 14, 1, 1, 1)
                + proj_halves(15, 16, 1, 2, 1)
                + proj_halves(17, 18, 1, 3, 1)
                # v units ~2-4 points before their at-ring wrap deadline
                # (#16p+4tb+RING-1), q(1,p) before #32/#48
                + proj_halves(19, 20, 2, 0, 0)
                + proj_halves(23, 24, 2, 1, 0)
                + proj_halves(26, 27, 2, 0, 1)
                + proj_halves(28, 29, 2, 2, 0)
                + proj_halves(29, 30, 0, 1, 0)
                + proj_halves(32, 33, 2, 3, 0)
                + proj_halves(35, 36, 2, 1, 1)
                + proj_halves(40, 41, 2, 2, 1)
                + proj_halves(44, 45, 2, 3, 1)
                + proj_halves(45, 46, 0, 1, 1)
                # q blocks 2-3: needed at exp #64/#80/#96/#112
                + proj_halves(58, 59, 0, 2, 0)
                + proj_halves(74, 75, 0, 2, 1)
                + proj_halves(90, 91, 0, 3, 0)
                + proj_halves(106, 107, 0, 3, 1)
            )
            units.sort(key=lambda u: u[0])
            ui = [0]

            def inject(point):
                while ui[0] < len(units) and units[ui[0]][0] <= point:
                    units[ui[0]][1]()
                    ui[0] += 1

            # ---- attention: global exp stream over (pair, i-chunk) sweeps
            SWEEPS = [(p, ic) for ic in range(NIC) for p in range(NPAIR)]

            pv_tiles = {}   # sweep index -> [pv_h0, pv_h1]
            pv_done = [0]   # count of pv units fully emitted

            sc_tiles = {}

            def emit_scores(e):
                p, ic = SWEEPS[e // JT]
                jt = e % JT
                assert (0, ic, p) in emitted_units, f"exp {e}: q({ic},{p}) not emitted"
                assert (1, jt // 4, p) in emitted_units, f"exp {e}: k({jt//4},{p}) not emitted"
                i0 = ic * IC
                sc = scp.tile([128, 2, IC], FP32, tag="sc", name="sc")
                for h in range(2):
                    nc.tensor.matmul(
                        sc[:, h, :],
                        k_t[h * DH : (h + 1) * DH, p, jt * 128 : (jt + 1) * 128],
                        q_t[h * DH : (h + 1) * DH, p, i0 : i0 + IC],
                        start=True, stop=True,
                    )
                sc_tiles[e] = sc

            def emit_act(e):
                sc = sc_tiles.pop(e)
                nc.scalar.activation(
                    out=at_t[:, e % RING, :],
                    in_=sc[:, :, :],
                    func=mybir.ActivationFunctionType.Exp,
                    scale=SCALE,
                    bias=shift_t[:, 0:1],
                )

            def emit_pv_unit(e):
                si, jt = e // JT, e % JT
                p, ic = SWEEPS[si]
                assert (2, jt // 4, p) in emitted_units, f"pv {e}: v({jt//4},{p}) not emitted"
                if jt == 0:
                    pv_tiles[si] = [
                        pvp.tile([128, IC], FP32, tag="pv", name=f"pv{si}_{h}")
                        for h in range(2)
                    ]
                pv = pv_tiles[si]
                r = e % RING
                for h in range(2):
                    nc.tensor.matmul(
                        pv[h][:, :],
                        v_t[:, p, jt, h, :],
                        at_t[:, r, h * IC : (h + 1) * IC],
                        start=(jt == 0), stop=(jt == JT - 1),
                    )
                if jt == JT - 1:
                    emit_norm(si)

            def emit_norm(si):
                p, ic = SWEEPS[si]
                i0 = ic * IC
                pv = pv_tiles.pop(si)
                for h in range(2):
                    densb = drainp.tile([DH, IC], FP32, tag="densb", name="densb")
                    nc.vector.tensor_copy(densb[:, :], pv[h][DH : 2 * DH, :])
                    rec = drainp.tile([DH, IC], FP32, tag="rec", name="rec")
                    nc.vector.reciprocal_approx_fast(rec[:, :], densb[:, :])
                    osb = outp.tile([DH, IC], FP32, tag="osb", name="osb")
                    nc.vector.tensor_mul(osb[:, :], pv[h][0:DH, :], rec[:, :])
                    nc.sync.dma_start(
                        out=out[
                            (2 * p + h) * DH : (2 * p + h + 1) * DH,
                            i0 : i0 + IC,
                        ],
                        in_=osb[:, :],
                    )

            def v_ready_pt(u):
                # injection point after which pv unit u's v block is in v_t
                p, _ = SWEEPS[u // JT]
                if u // JT >= 2:
                    return 0          # later sweeps reuse already-built v
                return v_h1_pt[p][(u % JT) // 4]

            # scores are emitted ONE exp ahead of their activation so the
            # PE finishes them well before the ScalarE queue needs them
            # (otherwise a point's proj/pv matmuls sit between ACT(e) and
            # scores(e+1) on the in-order PE queue and the ACT stream picks
            # up ~200ns of semaphore latency per exp)
            emit_scores(0)
            for e in range(NEXP):
                # the at ring must never wrap onto a slot whose pv read has
                # not even been emitted yet (program-order RAW/WAR safety)
                assert pv_done[0] > e - RING, (
                    f"at-ring wrap: exp {e} but pv_done only {pv_done[0]}"
                )
                if e + 1 < NEXP:
                    with nc.named_scope(f"S{e + 1}"):
                        emit_scores(e + 1)
                with nc.named_scope(f"A{e}"):
                    emit_act(e)
                if e == 2:
                    # pair-1 weight columns ride the scalar HWDGE queue
                    # (the gpsimd ring is full and they are needed by the
                    # pair-1 projections around exp #9-18)
                    nc.scalar.dma_start(
                        out=wq_t[:, :, 128:DC], in_=wq[:, :, 128:DC]
                    )
                elif e == 3:
                    nc.scalar.dma_start(
                        out=wk_t[:, :, 128:DC], in_=wk[:, :, 128:DC]
                    )
                elif 10 <= e <= 12:
                    # push q blocks 1-3 now: the k-path DMA has drained,
                    # and the trigger is only ~0.6us on the ACT queue
                    dma_x(nc.scalar, 0, e - 9)
                inject(e)
                # emit pv pairs whose v data is resident, trailing the exps
                # by >=4 (so a pv matmul never waits on a *recent* ACT and
                # thus never stalls the in-order PE queue), at most 2/point
                n_here = 0
                while (
                    pv_done[0] <= e - 4
                    and v_ready_pt(pv_done[0]) <= e
                    and n_here < 2
                ):
                    with nc.named_scope(f"P{pv_done[0]}"):
                        emit_pv_unit(pv_done[0])
                    pv_done[0] += 1
                    n_here += 1
            # drain remaining pv units + norms
            while pv_done[0] < NEXP:
                with nc.named_scope(f"P{pv_done[0]}"):
                    emit_pv_unit(pv_done[0])
                pv_done[0] += 1

    nc.finalize()
    return nc


_PROGRAM_CACHE = {}


def _get_program(S_, B_):
    assert (S_, B_) == (S, B)
    if "p" not in _PROGRAM_CACHE:
        _PROGRAM_CACHE["p"] = build_program()
    return _PROGRAM_CACHE["p"]


def make_in_maps(query, key, value, Wq, bq, Wk, bk, Wv, bv):
    S_, B_, D_ = query.shape
    assert (S_, B_, D_) == (S, B, D)

    def xt(a, b):
        # [S, B, D] -> [D, S] for batch b -> tiles [NTB, 128, KT, TB]
        aT = np.asarray(a[:, b, :], np.float32).T
        a4 = aT.reshape(KT, 128, NTB, TB).transpose(2, 1, 0, 3)
        return np.ascontiguousarray(a4).astype(NP_BF16)

    def wt_host(W, rows):
        # [DC rows, D] slice -> W.T [D, DC] -> [128, KT, DC] (partition-major)
        wT = np.asarray(W)[rows, :].T.astype(np.float32)
        w3 = wT.reshape(KT, 128, DC).transpose(1, 0, 2)
        return np.ascontiguousarray(w3).astype(NP_BF16)

    xq_b = [xt(query, b) for b in range(B)]
    xk_b = [xt(key, b) for b in range(B)]
    xv_b = [xt(value, b) for b in range(B)]

    in_maps = []
    for c in range(NCORES):
        b, hg = c // 4, c % 4
        rows = slice(hg * DC, (hg + 1) * DC)
        in_maps.append(
            {
                "xq": xq_b[b], "xk": xk_b[b], "xv": xv_b[b],
                "wq": wt_host(Wq, rows),
                "wk": wt_host(Wk, rows),
                "wv": wt_host(Wv, rows),
                "bqkv": np.ascontiguousarray(
                    np.stack(
                        [np.asarray(bq)[rows], np.asarray(bk)[rows], np.asarray(bv)[rows]],
                        axis=1,
                    ).reshape(NPAIR, 128, 3).transpose(1, 0, 2)
                ).astype(np.float32),
            }
        )
    return in_maps


def gather_output(results, S_, B_):
    full = np.empty((S, B, D), np.float32)
    for c in range(NCORES):
        b, hg = c // 4, c % 4
        o = np.asarray(results[c]["out"], np.float32)  # [DC, S]
        full[:, b, hg * DC : (hg + 1) * DC] = o.T
    return full


def kernel(query, key, value, Wq, bq, Wk, bk, Wv, bv):
    from concourse.bass_utils import run_bass_kernel_spmd

    S_, B_, _ = query.shape
    nc = _get_program(S_, B_)
    in_maps = make_in_maps(query, key, value, Wq, bq, Wk, bk, Wv, bv)
    res = run_bass_kernel_spmd(nc, in_maps, list(range(NCORES)))
    return gather_output(res.results, S_, B_)


# revision 34
# speedup vs baseline: 1.0066x; 1.0066x over previous
"""MultiHeadAttention kernel for Trainium2, 8-core hybrid batch x head sharding.

Problem: S=2048, B=2, D=1024, 16 heads of d=64 (batch_first=False).
Sharding: core c handles batch b=c//4 and head group hg=c%4 (4 heads =
256 output dims), processed as 2 "pairs" of 2 heads (a pair = 128
partitions = 2x64 head dims).

v2 schedule (vs v1): the exp stream is the critical engine (~147us of
ScalarE work); the v1 loss was ~46us of ACT gaps during the input DMA
window. Changes:
  - global exp order interleaves pairs per i-chunk:
    (p0,ic0),(p1,ic0),(p0,ic1),... so the first 32 exps need only
    q block 0; the DMA stream is ordered [wq,xq0,wk,xk0,k1,k2,k3,
    wv,v0..v3,q1,q2,q3] to match first-need times (k blocks gate the
    exp stream hardest, q blocks 1-3 are needed latest).
  - at_t ring enlarged to 20 slots so pv can trail the exps by up to
    18 j-tiles early on (v blocks arrive late in the DMA stream);
    the lag tapers back to 2 once all v data is resident.
  - PSUM: scores double-buffered 2x[128,1024] (4 banks), pv 2 banks,
    projection scratch 2 banks (warm chain shares its tag).
  - a PE warm chain bridges the initial DMA wait so projections run
    at 2.4GHz (HAM warm) from the start.
Everything else (ones-column softmax denominator inside pv, DVE-only
normalization, DMA-xbar v transpose) is unchanged from v1.
"""

import sys

if "/opt/trn_rl_repo" not in sys.path:
    sys.path.insert(0, "/opt/trn_rl_repo")

import numpy as np
import ml_dtypes

import concourse.bass as bass
import concourse.mybir as mybir
import concourse.tile as tile
from concourse import bacc

BF16 = mybir.dt.bfloat16
FP32 = mybir.dt.float32
FP8 = mybir.dt.float8e4
NP_BF16 = ml_dtypes.bfloat16

D = 1024
NHEAD = 16
DH = 64
NCORES = 8
S = 2048
B = 2
HPC = 4                      # heads per core
DC = HPC * DH                # per-core output dims = 256
NPAIR = 2                    # head pairs per core (128 dims each)
KT = D // 128                # contraction tiles = 8
TB = 512                     # token block for projections
NTB = S // TB                # 4
IC = 512                     # i-chunk width
NIC = S // IC                # 4
JT = S // 128                # j-tiles = 16
RING = 22                    # at_t ring slots
NEXP = NPAIR * NIC * JT      # 128 global exp units
SCALE = 1.0 / float(np.sqrt(DH))
# constant subtracted inside the exp so attention weights fit fp8e4m3
# (numerator and denominator of the softmax scale by the same e^-C, so
# the output is unchanged; scores*SCALE is ~N(8, 1.7) on this data)
EXP_SHIFT = -12.0


def build_program():
    nc = bacc.Bacc(
        "TRN2", target_bir_lowering=False, debug=False, num_devices=NCORES
    )
    xq = nc.dram_tensor("xq", [NTB, 128, KT, TB], BF16, kind="ExternalInput")
    xk = nc.dram_tensor("xk", [NTB, 128, KT, TB], BF16, kind="ExternalInput")
    xv = nc.dram_tensor("xv", [NTB, 128, KT, TB], BF16, kind="ExternalInput")
    wq = nc.dram_tensor("wq", [128, KT, DC], BF16, kind="ExternalInput")
    wk = nc.dram_tensor("wk", [128, KT, DC], BF16, kind="ExternalInput")
    wv = nc.dram_tensor("wv", [128, KT, DC], BF16, kind="ExternalInput")
    bqkv = nc.dram_tensor("bqkv", [128, NPAIR, 3], FP32, kind="ExternalInput")
    out = nc.dram_tensor("out", [DC, S], FP32, kind="ExternalOutput")

    with tile.TileContext(nc) as tc:
        with (
            tc.tile_pool(name="const", bufs=1) as constp,
            tc.tile_pool(name="xin", bufs=1) as xp,
            tc.tile_pool(name="qkv", bufs=1) as qkvp,
            tc.tile_pool(name="vstg", bufs=2) as vstgp,
            tc.tile_pool(name="attn", bufs=1) as atp,
            tc.tile_pool(name="outp", bufs=2) as outp,
            tc.tile_pool(name="drain", bufs=2) as drainp,
            tc.tile_pool(name="sc", bufs=2, space="PSUM") as scp,
            tc.tile_pool(name="prj", bufs=2, space="PSUM") as prjp,
            tc.tile_pool(name="pv", bufs=2, space="PSUM") as pvp,
        ):
            # ---- input loads, all on the SWDGE (gpsimd) queue, in
            # first-need order: prologue (wq,xq0,wk,xk0), then all k
            # blocks, then v, then the remaining q blocks.
            xq_t = xp.tile([128, NTB, KT, TB], BF16, tag="xq")
            xk_t = xp.tile([128, NTB, KT, TB], BF16, tag="xk")
            xv_t = xp.tile([128, NTB, KT, TB], BF16, tag="xv")
            xts = {0: xq_t, 1: xk_t, 2: xv_t}
            xsrc = {0: xq, 1: xk, 2: xv}
            wq_t = constp.tile([128, KT, DC], BF16, tag="wq")
            wk_t = constp.tile([128, KT, DC], BF16, tag="wk")
            wv_t = constp.tile([128, KT, DC], BF16, tag="wv")

            # Input stream: 12 transfers on the gpsimd SWDGE queue (its
            # descriptor ring is ~12 deep), in first-need order, with the
            # critical prefix minimized: only the pair-0 weight columns and
            # the first 128 k tokens come before the first exp can fire
            # (the whole-chip prefix traffic is what gates the start - all
            # 8 cores pull simultaneously). Pair-1 weights and q blocks 1-3
            # ride the scalar HWDGE queue, pushed between early exps; wv
            # rides sync.
            def dma_x(q, kind, tb):
                q.dma_start(out=xts[kind][:, tb], in_=xsrc[kind][tb])

            nc.gpsimd.dma_start(out=wq_t[:, :, 0:128], in_=wq[:, :, 0:128])
            dma_x(nc.gpsimd, 0, 0)
            nc.gpsimd.dma_start(out=wk_t[:, :, 0:128], in_=wk[:, :, 0:128])
            for tb in range(NTB):
                dma_x(nc.gpsimd, 1, tb)
            for tb in range(NTB):
                dma_x(nc.gpsimd, 2, tb)
            nc.sync.dma_start(out=wv_t[:], in_=wv[:, :, :])

            # ---- exp table preload: fire ACT_TABLE_LOAD during the DMA phase
            pre_in = constp.tile([128, 16], FP32, tag="prei")
            nc.vector.memset(pre_in[:], 0.0)
            pre_out = constp.tile([128, 16], BF16, tag="preo")
            nc.scalar.activation(
                out=pre_out[:], in_=pre_in[:],
                func=mybir.ActivationFunctionType.Exp, scale=1.0,
            )

            # ---- constants
            bqkv_t = constp.tile([128, NPAIR, 3], FP32, tag="bqkv")
            nc.sync.dma_start(out=bqkv_t[:], in_=bqkv[:, :, :])
            shift_t = constp.tile([128, 1], FP32, tag="shift")
            nc.vector.memset(shift_t[:], EXP_SHIFT)

            # ---- persistent activations
            q_t = qkvp.tile([128, NPAIR, S], BF16, tag="q")
            k_t = qkvp.tile([128, NPAIR, S], BF16, tag="k")
            vx_t = qkvp.tile([128, NPAIR, JT, 2 * DH], BF16, tag="vx")
            # per head: cols 0:64 = projected v, cols 64:128 = ones, so the
            # pv matmul replicates the softmax denominator across output
            # partitions 64:128. fp8 so pv can run in DoubleRow perf mode
            # (2 j-tiles per matmul - halves the pv matmul count).
            v_t = qkvp.tile([128, NPAIR, JT, 2, 2 * DH], BF16, tag="v")
            nc.vector.memset(v_t[:, :, :, :, DH : 2 * DH], 1.0)
            # attention-weights ring indexed by global exp unit % RING
            at_t = atp.tile([128, RING, 2 * IC], BF16, tag="at", bufs=1)

            wts = {0: wq_t, 1: wk_t, 2: wv_t}

            # ---- HAM warm chain: identical-weights matmuls chained on wq
            # keep the PE busy from wq arrival through the xk0 DMA wait so
            # the first projections run at 2.4GHz. Shares the proj scratch
            # tag (it is write-only; the first proj unit simply WARs on it).
            warm = prjp.tile([128, TB], FP32, tag="prj", name="warm")
            for _ in range(40):
                nc.tensor.matmul(
                    warm[:, 0:128], wq_t[:, 0, 0:128], wq_t[:, 0, 0:128],
                    start=True, stop=True,
                )

            proj_ps = {}
            emitted_units = set()   # (kind, tb, p) whose drain has been emitted

            def emit_proj_mm(kind, tb, p, half):
                if half == 1:
                    emitted_units.add((kind, tb, p))
                # one half of a projection's K-accumulation: 4 matmuls
                x_t = xts[kind]
                w_t = wts[kind]
                if half == 0:
                    proj_ps[(kind, tb, p)] = prjp.tile(
                        [128, TB], FP32, tag="prj", name=f"ps{kind}{tb}{p}"
                    )
                ps = proj_ps[(kind, tb, p)]
                for kt in range(half * 4, half * 4 + 4):
                    nc.tensor.matmul(
                        ps[:, :], w_t[:, kt, p * 128 : (p + 1) * 128],
                        x_t[:, tb, kt, :],
                        start=(kt == 0), stop=(kt == KT - 1),
                    )
                if half == 1:
                    bias = bqkv_t[:, p, kind : kind + 1].to_broadcast((128, TB))
                    if kind < 2:
                        dst = q_t if kind == 0 else k_t
                        nc.vector.tensor_add(
                            dst[:, p, tb * TB : (tb + 1) * TB], ps[:, :], bias
                        )
                    else:
                        vTt = vstgp.tile([128, TB], BF16, tag="vT", name="vT")
                        nc.vector.tensor_add(vTt[:, :], ps[:, :], bias)
                        j0 = tb * (TB // 128)
                        j1 = (tb + 1) * (TB // 128)
                        nc.sync.dma_start_transpose(vx_t[:, p, j0:j1, :], vTt[:, :])
                        nc.vector.tensor_copy(
                            v_t[:, p, j0:j1, :, 0:DH],
                            vx_t[:, p, j0:j1, :].rearrange("t j (h d) -> t j h d", h=2),
                        )
                    del proj_ps[(kind, tb, p)]

            def emit_proj(kind, tb, p):
                emit_proj_mm(kind, tb, p, 0)
                emit_proj_mm(kind, tb, p, 1)

            # prologue: q block 0 pair 0 (full), then k block 0 pair 0 in
            # two pieces - tokens 0:128 first (they unlock the first scores
            # tile as soon as the quarter-block xk0a DMA lands), then the
            # rest
            emit_proj(0, 0, 0)
            k00ps = prjp.tile([128, TB], FP32, tag="prj", name="k00ps")
            kbias = bqkv_t[:, 0, 1:2]
            for lo, hi in ((0, 128), (128, TB)):
                for kt in range(KT):
                    nc.tensor.matmul(
                        k00ps[:, lo:hi], wk_t[:, kt, 0:128],
                        xk_t[:, 0, kt, lo:hi],
                        start=(kt == 0), stop=(kt == KT - 1),
                    )
                nc.vector.tensor_add(
                    k_t[:, 0, lo:hi], k00ps[:, lo:hi],
                    kbias.to_broadcast((128, hi - lo)),
                )
            emitted_units.add((1, 0, 0))

            # ---- deadline-scheduled projection units, half-projection
            # granularity. point = global exp index (see SWEEPS below).
            def _scoped(name, fn):
                def g():
                    with nc.named_scope(name):
                        fn()
                return g

            def proj_halves(pt0, pt1, kind, tb, p):
                n = f"U{'qkv'[kind]}{tb}p{p}"
                return [
                    (pt0, _scoped(n + "a", lambda: emit_proj_mm(kind, tb, p, 0))),
                    (pt1, _scoped(n + "b", lambda: emit_proj_mm(kind, tb, p, 1))),
                ]

            # v_h1_pt[p][tb] = injection point of the v(tb,p) unit's second
            # half (whose drain writes v_t); pv units must be emitted AFTER
            # that point or they read uninitialized v_t (build asserts below)
            v_h1_pt = {0: [20, 24, 29, 33], 1: [27, 36, 41, 45]}
            units = (
                # k blocks 1-3 pair 0 (needed at exp #4tb) and all pair-1
                # k/q-block-0 projections go in the early, DMA-stalled
                # window where the PE has slack
                proj_halves(1, 2, 1, 1, 0)
                + proj_halves(4, 5, 1, 2, 0)
                + proj_halves(7, 8, 1, 3, 0)
                + proj_halves(9, 10, 0, 0, 1)
                + proj_halves(11, 12, 1, 0, 1)
                + proj_halves(13, 14, 1, 1, 1)
                + proj_halves(15, 16, 1, 2, 1)
                + proj_halves(17, 18, 1, 3, 1)
                # v units ~2-4 points before their at-ring wrap deadline
                # (#16p+4tb+RING-1), q(1,p) before #32/#48
                + proj_halves(19, 20, 2, 0, 0)
                + proj_halves(23, 24, 2, 1, 0)
                + proj_halves(26, 27, 2, 0, 1)
                + proj_halves(28, 29, 2, 2, 0)
                + proj_halves(29, 30, 0, 1, 0)
                + proj_halves(32, 33, 2, 3, 0)
                + proj_halves(35, 36, 2, 1, 1)
                + proj_halves(40, 41, 2, 2, 1)
                + proj_halves(44, 45, 2, 3, 1)
                + proj_halves(45, 46, 0, 1, 1)
                # q blocks 2-3: needed at exp #64/#80/#96/#112
                + proj_halves(58, 59, 0, 2, 0)
                + proj_halves(74, 75, 0, 2, 1)
                + proj_halves(90, 91, 0, 3, 0)
                + proj_halves(106, 107, 0, 3, 1)
            )
            units.sort(key=lambda u: u[0])
            ui = [0]

            def inject(point):
                while ui[0] < len(units) and units[ui[0]][0] <= point:
                    units[ui[0]][1]()
                    ui[0] += 1

            # ---- attention: global exp stream over (pair, i-chunk) sweeps
            SWEEPS = [(p, ic) for ic in range(NIC) for p in range(NPAIR)]

            pv_tiles = {}   # sweep index -> [pv_h0, pv_h1]
            pv_done = [0]   # count of pv units fully emitted

            sc_tiles = {}

            def emit_scores(e):
                p, ic = SWEEPS[e // JT]
                jt = e % JT
                assert (0, ic, p) in emitted_units, f"exp {e}: q({ic},{p}) not emitted"
                assert (1, jt // 4, p) in emitted_units, f"exp {e}: k({jt//4},{p}) not emitted"
                i0 = ic * IC
                sc = scp.tile([128, 2, IC], FP32, tag="sc", name="sc")
                for h in range(2):
                    nc.tensor.matmul(
                        sc[:, h, :],
                        k_t[h * DH : (h + 1) * DH, p, jt * 128 : (jt + 1) * 128],
                        q_t[h * DH : (h + 1) * DH, p, i0 : i0 + IC],
                        start=True, stop=True,
                    )
                sc_tiles[e] = sc

            def emit_act(e):
                sc = sc_tiles.pop(e)
                nc.scalar.activation(
                    out=at_t[:, e % RING, :],
                    in_=sc[:, :, :],
                    func=mybir.ActivationFunctionType.Exp,
                    scale=SCALE,
                    bias=shift_t[:, 0:1],
                )

            def emit_pv_unit(e):
                si, jt = e // JT, e % JT
                p, ic = SWEEPS[si]
                assert (2, jt // 4, p) in emitted_units, f"pv {e}: v({jt//4},{p}) not emitted"
                if jt == 0:
                    pv_tiles[si] = [
                        pvp.tile([128, IC], FP32, tag="pv", name=f"pv{si}_{h}")
                        for h in range(2)
                    ]
                pv = pv_tiles[si]
                r = e % RING
                for h in range(2):
                    nc.tensor.matmul(
                        pv[h][:, :],
                        v_t[:, p, jt, h, :],
                        at_t[:, r, h * IC : (h + 1) * IC],
                        start=(jt == 0), stop=(jt == JT - 1),
                    )
                if jt == JT - 1:
                    emit_norm(si)

            def emit_norm(si):
                p, ic = SWEEPS[si]
                i0 = ic * IC
                pv = pv_tiles.pop(si)
                for h in range(2):
                    densb = drainp.tile([DH, IC], FP32, tag="densb", name="densb")
                    nc.vector.tensor_copy(densb[:, :], pv[h][DH : 2 * DH, :])
                    rec = drainp.tile([DH, IC], FP32, tag="rec", name="rec")
                    nc.vector.reciprocal_approx_fast(rec[:, :], densb[:, :])
                    osb = outp.tile([DH, IC], FP32, tag="osb", name="osb")
                    nc.vector.tensor_mul(osb[:, :], pv[h][0:DH, :], rec[:, :])
                    nc.sync.dma_start(
                        out=out[
                            (2 * p + h) * DH : (2 * p + h + 1) * DH,
                            i0 : i0 + IC,
                        ],
                        in_=osb[:, :],
                    )

            def v_ready_pt(u):
                # injection point after which pv unit u's v block is in v_t
                p, _ = SWEEPS[u // JT]
                if u // JT >= 2:
                    return 0          # later sweeps reuse already-built v
                return v_h1_pt[p][(u % JT) // 4]

            # scores are emitted ONE exp ahead of their activation so the
            # PE finishes them well before the ScalarE queue needs them
            # (otherwise a point's proj/pv matmuls sit between ACT(e) and
            # scores(e+1) on the in-order PE queue and the ACT stream picks
            # up ~200ns of semaphore latency per exp)
            emit_scores(0)
            for e in range(NEXP):
                # the at ring must never wrap onto a slot whose pv read has
                # not even been emitted yet (program-order RAW/WAR safety)
                assert pv_done[0] > e - RING, (
                    f"at-ring wrap: exp {e} but pv_done only {pv_done[0]}"
                )
                if e + 1 < NEXP:
                    with nc.named_scope(f"S{e + 1}"):
                        emit_scores(e + 1)
                with nc.named_scope(f"A{e}"):
                    emit_act(e)
                if e == 2:
                    # pair-1 weight columns ride the scalar HWDGE queue
                    # (the gpsimd ring is full and they are needed by the
                    # pair-1 projections around exp #9-18)
                    nc.scalar.dma_start(
                        out=wq_t[:, :, 128:DC], in_=wq[:, :, 128:DC]
                    )
                elif e == 3:
                    nc.scalar.dma_start(
                        out=wk_t[:, :, 128:DC], in_=wk[:, :, 128:DC]
                    )
                elif 10 <= e <= 12:
                    # push q blocks 1-3 now: the k-path DMA has drained,
                    # and the trigger is only ~0.6us on the ACT queue
                    dma_x(nc.scalar, 0, e - 9)
                inject(e)
                # emit pv pairs whose v data is resident, trailing the exps
                # by >=4 (so a pv matmul never waits on a *recent* ACT and
                # thus never stalls the in-order PE queue), at most 2/point
                n_here = 0
                while (
                    pv_done[0] <= e - 4
                    and v_ready_pt(pv_done[0]) <= e
                    and n_here < 2
                ):
                    with nc.named_scope(f"P{pv_done[0]}"):
                        emit_pv_unit(pv_done[0])
                    pv_done[0] += 1
                    n_here += 1
            # drain remaining pv units + norms
            while pv_done[0] < NEXP:
                with nc.named_scope(f"P{pv_done[0]}"):
                    emit_pv_unit(pv_done[0])
                pv_done[0] += 1

    nc.finalize()
    return nc


_PROGRAM_CACHE = {}


def _get_program(S_, B_):
    assert (S_, B_) == (S, B)
    if "p" not in _PROGRAM_CACHE:
        _PROGRAM_CACHE["p"] = build_program()
    return _PROGRAM_CACHE["p"]


def make_in_maps(query, key, value, Wq, bq, Wk, bk, Wv, bv):
    S_, B_, D_ = query.shape
    assert (S_, B_, D_) == (S, B, D)

    def xt(a, b):
        # [S, B, D] -> [D, S] for batch b -> tiles [NTB, 128, KT, TB]
        aT = np.asarray(a[:, b, :], np.float32).T
        a4 = aT.reshape(KT, 128, NTB, TB).transpose(2, 1, 0, 3)
        return np.ascontiguousarray(a4).astype(NP_BF16)

    def wt_host(W, rows):
        # [DC rows, D] slice -> W.T [D, DC] -> [128, KT, DC] (partition-major)
        wT = np.asarray(W)[rows, :].T.astype(np.float32)
        w3 = wT.reshape(KT, 128, DC).transpose(1, 0, 2)
        return np.ascontiguousarray(w3).astype(NP_BF16)

    xq_b = [xt(query, b) for b in range(B)]
    xk_b = [xt(key, b) for b in range(B)]
    xv_b = [xt(value, b) for b in range(B)]

    in_maps = []
    for c in range(NCORES):
        b, hg = c // 4, c % 4
        rows = slice(hg * DC, (hg + 1) * DC)
        in_maps.append(
            {
                "xq": xq_b[b], "xk": xk_b[b], "xv": xv_b[b],
                "wq": wt_host(Wq, rows),
                "wk": wt_host(Wk, rows),
                "wv": wt_host(Wv, rows),
                "bqkv": np.ascontiguousarray(
                    np.stack(
                        [np.asarray(bq)[rows], np.asarray(bk)[rows], np.asarray(bv)[rows]],
                        axis=1,
                    ).reshape(NPAIR, 128, 3).transpose(1, 0, 2)
                ).astype(np.float32),
            }
        )
    return in_maps


def gather_output(results, S_, B_):
    full = np.empty((S, B, D), np.float32)
    for c in range(NCORES):
        b, hg = c // 4, c % 4
        o = np.asarray(results[c]["out"], np.float32)  # [DC, S]
        full[:, b, hg * DC : (hg + 1) * DC] = o.T
    return full


def kernel(query, key, value, Wq, bq, Wk, bk, Wv, bv):
    from concourse.bass_utils import run_bass_kernel_spmd

    S_, B_, _ = query.shape
    nc = _get_program(S_, B_)
    in_maps = make_in_maps(query, key, value, Wq, bq, Wk, bk, Wv, bv)
    res = run_bass_kernel_spmd(nc, in_maps, list(range(NCORES)))
    return gather_output(res.results, S_, B_)


# revision 35
# speedup vs baseline: 1.0081x; 1.0015x over previous
"""MultiHeadAttention kernel for Trainium2, 8-core hybrid batch x head sharding.

Problem: S=2048, B=2, D=1024, 16 heads of d=64 (batch_first=False).
Sharding: core c handles batch b=c//4 and head group hg=c%4 (4 heads =
256 output dims), processed as 2 "pairs" of 2 heads (a pair = 128
partitions = 2x64 head dims).

v2 schedule (vs v1): the exp stream is the critical engine (~147us of
ScalarE work); the v1 loss was ~46us of ACT gaps during the input DMA
window. Changes:
  - global exp order interleaves pairs per i-chunk:
    (p0,ic0),(p1,ic0),(p0,ic1),... so the first 32 exps need only
    q block 0; the DMA stream is ordered [wq,xq0,wk,xk0,k1,k2,k3,
    wv,v0..v3,q1,q2,q3] to match first-need times (k blocks gate the
    exp stream hardest, q blocks 1-3 are needed latest).
  - at_t ring enlarged to 20 slots so pv can trail the exps by up to
    18 j-tiles early on (v blocks arrive late in the DMA stream);
    the lag tapers back to 2 once all v data is resident.
  - PSUM: scores double-buffered 2x[128,1024] (4 banks), pv 2 banks,
    projection scratch 2 banks (warm chain shares its tag).
  - a PE warm chain bridges the initial DMA wait so projections run
    at 2.4GHz (HAM warm) from the start.
Everything else (ones-column softmax denominator inside pv, DVE-only
normalization, DMA-xbar v transpose) is unchanged from v1.
"""

import sys

if "/opt/trn_rl_repo" not in sys.path:
    sys.path.insert(0, "/opt/trn_rl_repo")

import numpy as np
import ml_dtypes

import concourse.bass as bass
import concourse.mybir as mybir
import concourse.tile as tile
from concourse import bacc

BF16 = mybir.dt.bfloat16
FP32 = mybir.dt.float32
FP8 = mybir.dt.float8e4
NP_BF16 = ml_dtypes.bfloat16

D = 1024
NHEAD = 16
DH = 64
NCORES = 8
S = 2048
B = 2
HPC = 4                      # heads per core
DC = HPC * DH                # per-core output dims = 256
NPAIR = 2                    # head pairs per core (128 dims each)
KT = D // 128                # contraction tiles = 8
TB = 512                     # token block for projections
NTB = S // TB                # 4
IC = 512                     # i-chunk width
NIC = S // IC                # 4
JT = S // 128                # j-tiles = 16
RING = 22                    # at_t ring slots
NEXP = NPAIR * NIC * JT      # 128 global exp units
SCALE = 1.0 / float(np.sqrt(DH))
# constant subtracted inside the exp so attention weights fit fp8e4m3
# (numerator and denominator of the softmax scale by the same e^-C, so
# the output is unchanged; scores*SCALE is ~N(8, 1.7) on this data)
EXP_SHIFT = -12.0


def build_program():
    nc = bacc.Bacc(
        "TRN2", target_bir_lowering=False, debug=False, num_devices=NCORES
    )
    xq = nc.dram_tensor("xq", [NTB, 128, KT, TB], BF16, kind="ExternalInput")
    xk = nc.dram_tensor("xk", [NTB, 128, KT, TB], BF16, kind="ExternalInput")
    xv = nc.dram_tensor("xv", [NTB, 128, KT, TB], BF16, kind="ExternalInput")
    wq = nc.dram_tensor("wq", [128, KT, DC], BF16, kind="ExternalInput")
    wk = nc.dram_tensor("wk", [128, KT, DC], BF16, kind="ExternalInput")
    wv = nc.dram_tensor("wv", [128, KT, DC], BF16, kind="ExternalInput")
    bqkv = nc.dram_tensor("bqkv", [128, NPAIR, 3], FP32, kind="ExternalInput")
    out = nc.dram_tensor("out", [DC, S], FP32, kind="ExternalOutput")

    with tile.TileContext(nc) as tc:
        with (
            tc.tile_pool(name="const", bufs=1) as constp,
            tc.tile_pool(name="xin", bufs=1) as xp,
            tc.tile_pool(name="qkv", bufs=1) as qkvp,
            tc.tile_pool(name="vstg", bufs=2) as vstgp,
            tc.tile_pool(name="attn", bufs=1) as atp,
            tc.tile_pool(name="outp", bufs=2) as outp,
            tc.tile_pool(name="drain", bufs=2) as drainp,
            tc.tile_pool(name="sc", bufs=2, space="PSUM") as scp,
            tc.tile_pool(name="prj", bufs=2, space="PSUM") as prjp,
            tc.tile_pool(name="pv", bufs=2, space="PSUM") as pvp,
        ):
            # ---- input loads, all on the SWDGE (gpsimd) queue, in
            # first-need order: prologue (wq,xq0,wk,xk0), then all k
            # blocks, then v, then the remaining q blocks.
            xq_t = xp.tile([128, NTB, KT, TB], BF16, tag="xq")
            xk_t = xp.tile([128, NTB, KT, TB], BF16, tag="xk")
            xv_t = xp.tile([128, NTB, KT, TB], BF16, tag="xv")
            xts = {0: xq_t, 1: xk_t, 2: xv_t}
            xsrc = {0: xq, 1: xk, 2: xv}
            wq_t = constp.tile([128, KT, DC], BF16, tag="wq")
            wk_t = constp.tile([128, KT, DC], BF16, tag="wk")
            wv_t = constp.tile([128, KT, DC], BF16, tag="wv")

            # Input stream: 12 transfers on the gpsimd SWDGE queue (its
            # descriptor ring is ~12 deep), in first-need order, with the
            # critical prefix minimized: only the pair-0 weight columns and
            # the first 128 k tokens come before the first exp can fire
            # (the whole-chip prefix traffic is what gates the start - all
            # 8 cores pull simultaneously). Pair-1 weights and q blocks 1-3
            # ride the scalar HWDGE queue, pushed between early exps; wv
            # rides sync.
            def dma_x(q, kind, tb):
                q.dma_start(out=xts[kind][:, tb], in_=xsrc[kind][tb])

            nc.gpsimd.dma_start(out=wq_t[:, :, 0:128], in_=wq[:, :, 0:128])
            dma_x(nc.gpsimd, 0, 0)
            nc.gpsimd.dma_start(out=wk_t[:, :, 0:128], in_=wk[:, :, 0:128])
            for tb in range(NTB):
                dma_x(nc.gpsimd, 1, tb)
            for tb in range(NTB):
                dma_x(nc.gpsimd, 2, tb)
            nc.sync.dma_start(out=wv_t[:], in_=wv[:, :, :])

            # ---- exp table preload: fire ACT_TABLE_LOAD during the DMA phase
            pre_in = constp.tile([128, 16], FP32, tag="prei")
            nc.vector.memset(pre_in[:], 0.0)
            pre_out = constp.tile([128, 16], BF16, tag="preo")
            nc.scalar.activation(
                out=pre_out[:], in_=pre_in[:],
                func=mybir.ActivationFunctionType.Exp, scale=1.0,
            )

            # ---- constants
            bqkv_t = constp.tile([128, NPAIR, 3], FP32, tag="bqkv")
            nc.sync.dma_start(out=bqkv_t[:], in_=bqkv[:, :, :])
            shift_t = constp.tile([128, 1], FP32, tag="shift")
            nc.vector.memset(shift_t[:], EXP_SHIFT)

            # ---- persistent activations
            q_t = qkvp.tile([128, NPAIR, S], BF16, tag="q")
            k_t = qkvp.tile([128, NPAIR, S], BF16, tag="k")
            vx_t = qkvp.tile([128, NPAIR, JT, 2 * DH], BF16, tag="vx")
            # per head: cols 0:64 = projected v, cols 64:128 = ones, so the
            # pv matmul replicates the softmax denominator across output
            # partitions 64:128. fp8 so pv can run in DoubleRow perf mode
            # (2 j-tiles per matmul - halves the pv matmul count).
            v_t = qkvp.tile([128, NPAIR, JT, 2, 2 * DH], BF16, tag="v")
            nc.vector.memset(v_t[:, :, :, :, DH : 2 * DH], 1.0)
            # attention-weights ring indexed by global exp unit % RING
            at_t = atp.tile([128, RING, 2 * IC], BF16, tag="at", bufs=1)

            wts = {0: wq_t, 1: wk_t, 2: wv_t}

            # ---- HAM warm chain: identical-weights matmuls chained on wq
            # keep the PE busy from wq arrival through the xk0 DMA wait so
            # the first projections run at 2.4GHz. Shares the proj scratch
            # tag (it is write-only; the first proj unit simply WARs on it).
            warm = prjp.tile([128, TB], FP32, tag="prj", name="warm")
            for _ in range(40):
                nc.tensor.matmul(
                    warm[:, 0:128], wq_t[:, 0, 0:128], wq_t[:, 0, 0:128],
                    start=True, stop=True,
                )

            proj_ps = {}
            emitted_units = set()   # (kind, tb, p) whose drain has been emitted

            def emit_proj_mm(kind, tb, p, half):
                if half == 1:
                    emitted_units.add((kind, tb, p))
                # one half of a projection's K-accumulation: 4 matmuls
                x_t = xts[kind]
                w_t = wts[kind]
                if half == 0:
                    proj_ps[(kind, tb, p)] = prjp.tile(
                        [128, TB], FP32, tag="prj", name=f"ps{kind}{tb}{p}"
                    )
                ps = proj_ps[(kind, tb, p)]
                for kt in range(half * 4, half * 4 + 4):
                    nc.tensor.matmul(
                        ps[:, :], w_t[:, kt, p * 128 : (p + 1) * 128],
                        x_t[:, tb, kt, :],
                        start=(kt == 0), stop=(kt == KT - 1),
                    )
                if half == 1:
                    bias = bqkv_t[:, p, kind : kind + 1].to_broadcast((128, TB))
                    if kind < 2:
                        dst = q_t if kind == 0 else k_t
                        nc.vector.tensor_add(
                            dst[:, p, tb * TB : (tb + 1) * TB], ps[:, :], bias
                        )
                    else:
                        vTt = vstgp.tile([128, TB], BF16, tag="vT", name="vT")
                        nc.vector.tensor_add(vTt[:, :], ps[:, :], bias)
                        j0 = tb * (TB // 128)
                        j1 = (tb + 1) * (TB // 128)
                        nc.sync.dma_start_transpose(vx_t[:, p, j0:j1, :], vTt[:, :])
                        nc.vector.tensor_copy(
                            v_t[:, p, j0:j1, :, 0:DH],
                            vx_t[:, p, j0:j1, :].rearrange("t j (h d) -> t j h d", h=2),
                        )
                    del proj_ps[(kind, tb, p)]

            def emit_proj(kind, tb, p):
                emit_proj_mm(kind, tb, p, 0)
                emit_proj_mm(kind, tb, p, 1)

            # prologue: q block 0 pair 0 (full), then k block 0 pair 0 in
            # two pieces - tokens 0:128 first (they unlock the first scores
            # tile as soon as the quarter-block xk0a DMA lands), then the
            # rest
            emit_proj(0, 0, 0)
            k00ps = prjp.tile([128, TB], FP32, tag="prj", name="k00ps")
            kbias = bqkv_t[:, 0, 1:2]
            for lo, hi in ((0, 128), (128, TB)):
                for kt in range(KT):
                    nc.tensor.matmul(
                        k00ps[:, lo:hi], wk_t[:, kt, 0:128],
                        xk_t[:, 0, kt, lo:hi],
                        start=(kt == 0), stop=(kt == KT - 1),
                    )
                nc.vector.tensor_add(
                    k_t[:, 0, lo:hi], k00ps[:, lo:hi],
                    kbias.to_broadcast((128, hi - lo)),
                )
            emitted_units.add((1, 0, 0))

            # ---- deadline-scheduled projection units, half-projection
            # granularity. point = global exp index (see SWEEPS below).
            def _scoped(name, fn):
                def g():
                    with nc.named_scope(name):
                        fn()
                return g

            def proj_halves(pt0, pt1, kind, tb, p):
                n = f"U{'qkv'[kind]}{tb}p{p}"
                return [
                    (pt0, _scoped(n + "a", lambda: emit_proj_mm(kind, tb, p, 0))),
                    (pt1, _scoped(n + "b", lambda: emit_proj_mm(kind, tb, p, 1))),
                ]

            # v_h1_pt[p][tb] = injection point of the v(tb,p) unit's second
            # half (whose drain writes v_t); pv units must be emitted AFTER
            # that point or they read uninitialized v_t (build asserts below)
            v_h1_pt = {0: [20, 23, 28, 32], 1: [26, 35, 39, 43]}
            units = (
                # k blocks 1-3 pair 0 (needed at exp #4tb) and all pair-1
                # k/q-block-0 projections go in the early, DMA-stalled
                # window where the PE has slack
                proj_halves(1, 2, 1, 1, 0)
                + proj_halves(4, 5, 1, 2, 0)
                + proj_halves(7, 8, 1, 3, 0)
                + proj_halves(9, 10, 0, 0, 1)
                + proj_halves(11, 12, 1, 0, 1)
                + proj_halves(13, 14, 1, 1, 1)
                + proj_halves(15, 16, 1, 2, 1)
                + proj_halves(17, 18, 1, 3, 1)
                # v units ~2-4 points before their at-ring wrap deadline
                # (#16p+4tb+RING-1), q(1,p) before #32/#48
                + proj_halves(19, 20, 2, 0, 0)
                + proj_halves(22, 23, 2, 1, 0)
                + proj_halves(25, 26, 2, 0, 1)
                + proj_halves(27, 28, 2, 2, 0)
                + proj_halves(29, 30, 0, 1, 0)
                + proj_halves(31, 32, 2, 3, 0)
                + proj_halves(34, 35, 2, 1, 1)
                + proj_halves(38, 39, 2, 2, 1)
                + proj_halves(42, 43, 2, 3, 1)
                + proj_halves(45, 46, 0, 1, 1)
                # q blocks 2-3: needed at exp #64/#80/#96/#112
                + proj_halves(56, 58, 0, 2, 0)
                + proj_halves(72, 74, 0, 2, 1)
                + proj_halves(88, 90, 0, 3, 0)
                + proj_halves(104, 106, 0, 3, 1)
            )
            units.sort(key=lambda u: u[0])
            ui = [0]

            def inject(point):
                while ui[0] < len(units) and units[ui[0]][0] <= point:
                    units[ui[0]][1]()
                    ui[0] += 1

            # ---- attention: global exp stream over (pair, i-chunk) sweeps
            SWEEPS = [(p, ic) for ic in range(NIC) for p in range(NPAIR)]

            pv_tiles = {}   # sweep index -> [pv_h0, pv_h1]
            pv_done = [0]   # count of pv units fully emitted

            sc_tiles = {}

            def emit_scores(e):
                p, ic = SWEEPS[e // JT]
                jt = e % JT
                assert (0, ic, p) in emitted_units, f"exp {e}: q({ic},{p}) not emitted"
                assert (1, jt // 4, p) in emitted_units, f"exp {e}: k({jt//4},{p}) not emitted"
                i0 = ic * IC
                sc = scp.tile([128, 2, IC], FP32, tag="sc", name="sc")
                for h in range(2):
                    nc.tensor.matmul(
                        sc[:, h, :],
                        k_t[h * DH : (h + 1) * DH, p, jt * 128 : (jt + 1) * 128],
                        q_t[h * DH : (h + 1) * DH, p, i0 : i0 + IC],
                        start=True, stop=True,
                    )
                sc_tiles[e] = sc

            def emit_act(e):
                sc = sc_tiles.pop(e)
                nc.scalar.activation(
                    out=at_t[:, e % RING, :],
                    in_=sc[:, :, :],
                    func=mybir.ActivationFunctionType.Exp,
                    scale=SCALE,
                    bias=shift_t[:, 0:1],
                )

            def emit_pv_unit(e):
                si, jt = e // JT, e % JT
                p, ic = SWEEPS[si]
                assert (2, jt // 4, p) in emitted_units, f"pv {e}: v({jt//4},{p}) not emitted"
                if jt == 0:
                    pv_tiles[si] = [
                        pvp.tile([128, IC], FP32, tag="pv", name=f"pv{si}_{h}")
                        for h in range(2)
                    ]
                pv = pv_tiles[si]
                r = e % RING
                for h in range(2):
                    nc.tensor.matmul(
                        pv[h][:, :],
                        v_t[:, p, jt, h, :],
                        at_t[:, r, h * IC : (h + 1) * IC],
                        start=(jt == 0), stop=(jt == JT - 1),
                    )
                if jt == JT - 1:
                    emit_norm(si)

            def emit_norm(si):
                p, ic = SWEEPS[si]
                i0 = ic * IC
                pv = pv_tiles.pop(si)
                for h in range(2):
                    densb = drainp.tile([DH, IC], FP32, tag="densb", name="densb")
                    nc.vector.tensor_copy(densb[:, :], pv[h][DH : 2 * DH, :])
                    rec = drainp.tile([DH, IC], FP32, tag="rec", name="rec")
                    nc.vector.reciprocal_approx_fast(rec[:, :], densb[:, :])
                    osb = outp.tile([DH, IC], FP32, tag="osb", name="osb")
                    nc.vector.tensor_mul(osb[:, :], pv[h][0:DH, :], rec[:, :])
                    nc.sync.dma_start(
                        out=out[
                            (2 * p + h) * DH : (2 * p + h + 1) * DH,
                            i0 : i0 + IC,
                        ],
                        in_=osb[:, :],
                    )

            def v_ready_pt(u):
                # injection point after which pv unit u's v block is in v_t
                p, _ = SWEEPS[u // JT]
                if u // JT >= 2:
                    return 0          # later sweeps reuse already-built v
                return v_h1_pt[p][(u % JT) // 4]

            # scores are emitted ONE exp ahead of their activation so the
            # PE finishes them well before the ScalarE queue needs them
            # (otherwise a point's proj/pv matmuls sit between ACT(e) and
            # scores(e+1) on the in-order PE queue and the ACT stream picks
            # up ~200ns of semaphore latency per exp)
            emit_scores(0)
            for e in range(NEXP):
                # the at ring must never wrap onto a slot whose pv read has
                # not even been emitted yet (program-order RAW/WAR safety)
                assert pv_done[0] > e - RING, (
                    f"at-ring wrap: exp {e} but pv_done only {pv_done[0]}"
                )
                if e + 1 < NEXP:
                    with nc.named_scope(f"S{e + 1}"):
                        emit_scores(e + 1)
                with nc.named_scope(f"A{e}"):
                    emit_act(e)
                if e == 2:
                    # pair-1 weight columns ride the scalar HWDGE queue
                    # (the gpsimd ring is full and they are needed by the
                    # pair-1 projections around exp #9-18)
                    nc.scalar.dma_start(
                        out=wq_t[:, :, 128:DC], in_=wq[:, :, 128:DC]
                    )
                elif e == 3:
                    nc.scalar.dma_start(
                        out=wk_t[:, :, 128:DC], in_=wk[:, :, 128:DC]
                    )
                elif 10 <= e <= 12:
                    # push q blocks 1-3 now: the k-path DMA has drained,
                    # and the trigger is only ~0.6us on the ACT queue
                    dma_x(nc.scalar, 0, e - 9)
                inject(e)
                # emit pv pairs whose v data is resident, trailing the exps
                # by >=4 (so a pv matmul never waits on a *recent* ACT and
                # thus never stalls the in-order PE queue), at most 2/point
                n_here = 0
                while (
                    pv_done[0] <= e - (4 if e < 120 else 2)
                    and v_ready_pt(pv_done[0]) <= e
                    and n_here < 2
                ):
                    with nc.named_scope(f"P{pv_done[0]}"):
                        emit_pv_unit(pv_done[0])
                    pv_done[0] += 1
                    n_here += 1
            # drain remaining pv units + norms
            while pv_done[0] < NEXP:
                with nc.named_scope(f"P{pv_done[0]}"):
                    emit_pv_unit(pv_done[0])
                pv_done[0] += 1

    nc.finalize()
    return nc


_PROGRAM_CACHE = {}


def _get_program(S_, B_):
    assert (S_, B_) == (S, B)
    if "p" not in _PROGRAM_CACHE:
        _PROGRAM_CACHE["p"] = build_program()
    return _PROGRAM_CACHE["p"]


def make_in_maps(query, key, value, Wq, bq, Wk, bk, Wv, bv):
    S_, B_, D_ = query.shape
    assert (S_, B_, D_) == (S, B, D)

    def xt(a, b):
        # [S, B, D] -> [D, S] for batch b -> tiles [NTB, 128, KT, TB]
        aT = np.asarray(a[:, b, :], np.float32).T
        a4 = aT.reshape(KT, 128, NTB, TB).transpose(2, 1, 0, 3)
        return np.ascontiguousarray(a4).astype(NP_BF16)

    def wt_host(W, rows):
        # [DC rows, D] slice -> W.T [D, DC] -> [128, KT, DC] (partition-major)
        wT = np.asarray(W)[rows, :].T.astype(np.float32)
        w3 = wT.reshape(KT, 128, DC).transpose(1, 0, 2)
        return np.ascontiguousarray(w3).astype(NP_BF16)

    xq_b = [xt(query, b) for b in range(B)]
    xk_b = [xt(key, b) for b in range(B)]
    xv_b = [xt(value, b) for b in range(B)]

    in_maps = []
    for c in range(NCORES):
        b, hg = c // 4, c % 4
        rows = slice(hg * DC, (hg + 1) * DC)
        in_maps.append(
            {
                "xq": xq_b[b], "xk": xk_b[b], "xv": xv_b[b],
                "wq": wt_host(Wq, rows),
                "wk": wt_host(Wk, rows),
                "wv": wt_host(Wv, rows),
                "bqkv": np.ascontiguousarray(
                    np.stack(
                        [np.asarray(bq)[rows], np.asarray(bk)[rows], np.asarray(bv)[rows]],
                        axis=1,
                    ).reshape(NPAIR, 128, 3).transpose(1, 0, 2)
                ).astype(np.float32),
            }
        )
    return in_maps


def gather_output(results, S_, B_):
    full = np.empty((S, B, D), np.float32)
    for c in range(NCORES):
        b, hg = c // 4, c % 4
        o = np.asarray(results[c]["out"], np.float32)  # [DC, S]
        full[:, b, hg * DC : (hg + 1) * DC] = o.T
    return full


def kernel(query, key, value, Wq, bq, Wk, bk, Wv, bv):
    from concourse.bass_utils import run_bass_kernel_spmd

    S_, B_, _ = query.shape
    nc = _get_program(S_, B_)
    in_maps = make_in_maps(query, key, value, Wq, bq, Wk, bk, Wv, bv)
    res = run_bass_kernel_spmd(nc, in_maps, list(range(NCORES)))
    return gather_output(res.results, S_, B_)
